# revision 19
# baseline (speedup 1.0000x reference)
"""Bilateral denoiser (11x11 window, sigma=2) on 8 Trainium2 NeuronCores.

Math (per output pixel p, tap offset t=(dy,dx), d2 = dy^2+dx^2):
    w_t = exp(128*ln(dot(nrm_p, nrm_{p+t})) - |z_{p+t}-z_p|*min(1/(dz_p*c), 1e4)
              - d2/8)
        = clip(dot,0,1)^128 * exp(-|dz|/max(dz*sqrt(d2), 1e-4)) * exp(-d2/8)
    out = sum_t w_t * col_{p+t} / sum_t w_t     (center tap has w=1)

Sharding: H=1080 rows -> 10 tile rows of exactly 108 output rows, each
computed from a 128-row input tile (+-10-row halo in the partition dim).
Every core gets tile-row i full-width plus one 480-wide strip of tile-rows
8/9, so all cores run an identical (SPMD) program on identical shapes.

Symmetry: dot(n_p, n_{p+t}) and |z_{p+t}-z_p| are symmetric in (p, p+t), so
each pair {t, -t} shares one dot/|dz| plane computed on 108+dy extended rows;
the +t member reads it through a partition-shifted DMA copy (engine access
patterns must start at partition 0 -- quadrant rule -- so shifts go via DMA).

Host side only pads/deinterleaves/slices (layout, no math); all arithmetic
runs on-device: DVE tensor ops + ScalarE Ln/Exp/Abs (one act table set).
"""
import math

import numpy as np

import concourse.bacc as bacc
import concourse.tile as tile
from concourse import mybir
from concourse.bass_utils import run_bass_kernel_spmd

F32 = mybir.dt.float32
F16 = mybir.dt.float16
AF = mybir.ActivationFunctionType
OP = mybir.AluOpType

RAD = 5
H, W = 1080, 1920
TILE_OUT = 108            # output rows per 128-partition tile (2*RAD halo x2)
VPAD = 2 * RAD            # vertical halo rows above/below each tile
RM_CLAMP = 1.0 / (128.0 * 1e-4)

# f32 plane order: 0:3 = normal(xyz), 3 = z, 4 = dz;  colors ride separately
# as fp16 planes (halves the DVE cost of the color multiply/accumulate).
PLANE_PERM = [3, 4, 5, 6, 7]  # input channels -> f32 planes
COL_PERM = [0, 1, 2]          # input channels -> f16 color planes


def tap_classes(rad=RAD):
    cls = {}
    for dy in range(-rad, rad + 1):
        for dx in range(-rad, rad + 1):
            if dy == 0 and dx == 0:
                continue
            cls.setdefault(dy * dy + dx * dx, []).append((dy, dx))
    return sorted(cls.items())


def _pairs(classes):
    """Pairs {(dy,dx), (-dy,-dx)} grouped by dy >= 0; rep has dy>0 or
    (dy==0 and dx>0). Returns {dy: [dx,...]} honoring the class subset."""
    tap_set = {t for _, taps in classes for t in taps}
    groups = {}
    for dy in range(0, RAD + 1):
        dxs = []
        for dx in range(-RAD, RAD + 1):
            if dy == 0 and dx <= 0:
                continue
            if (dy, dx) in tap_set:
                dxs.append(dx)
        if dxs:
            # order by |dx| so +-dx neighbors share the rm plane
            dxs.sort(key=lambda d: (abs(d), -d))
            groups[dy] = dxs
    return groups


def _emit_member(nc, pools, ctx, lnu_ap, azd_ap, col_ap, rm_ap, bias_ap,
                 gacc, gaccw, first):
    """Accumulate one tap given its aligned ln(dot), |dz|, color APs.

    Color contributions go into the per-dy-group fp16 partial ``gacc``
    (small magnitudes -> small fp16 rounding); the first member writes it
    directly."""
    t1p, t3p = pools
    op, n_out, accw, acc3 = ctx
    # t1 = |dz|*rm in fp16 (2x DVE mode); x = lnu - t1 back in f32 so the
    # 128x-amplified exponent keeps full precision.
    t1 = t1p.tile([128, n_out], F16, tag="t1h", bufs=3)
    nc.vector.tensor_tensor(out=t1[op, :], in0=azd_ap, in1=rm_ap, op=OP.mult)
    xx = t1p.tile([128, n_out], F32, tag="s", bufs=3)
    nc.vector.tensor_tensor(out=xx[op, :], in0=lnu_ap, in1=t1[op, :],
                            op=OP.subtract)
    # w in fp16: the same rounded w feeds numerator and denominator, so the
    # rounding largely cancels in the final ratio.
    wt = t1p.tile([128, n_out], F16, tag="w", bufs=3)
    nc.scalar.activation(out=wt[op, :], in_=xx[op, :], func=AF.Exp,
                         scale=128.0, bias=bias_ap)
    nc.vector.tensor_tensor(out=gaccw[op, :], in0=gaccw[op, :],
                            in1=wt[op, :], op=OP.add)
    w_b = wt[op, None, :].to_broadcast((TILE_OUT, 3, n_out))
    if first:
        nc.vector.tensor_tensor(out=gacc[op, :, :], in0=col_ap, in1=w_b,
                                op=OP.mult)
    else:
        wc = t3p.tile([128, 3, n_out], F16, tag="p3c", bufs=2)
        nc.vector.tensor_tensor(out=wc[op, :, :], in0=col_ap, in1=w_b,
                                op=OP.mult)
        nc.vector.tensor_tensor(out=gacc[op, :, :], in0=gacc[op, :, :],
                                in1=wc[op, :, :], op=OP.add)


def _emit_item(nc, pools, src_ap, srcc_ap, dst_ap, n_in, classes, bias_tile,
               cls_idx):
    """One work item: input planes [8,128,n_in] -> output [108,3,n_in-10]."""
    n_out = n_in - 2 * RAD
    inp, shp, t3p, t1p, accp, rmp = pools
    op = slice(0, TILE_OUT)            # output rows = in_t rows [10,118)
    of = slice(RAD, RAD + n_out)       # center free columns within n_in

    pair_groups = _pairs(classes)

    in_t = inp.tile([128, 5, n_in], F32, tag="in", bufs=2)
    nc.sync.dma_start(out=in_t[:, :, :], in_=src_ap)

    # --- normalize normals in place: n *= (max(|n|^2,1e-20))^-0.5
    pr3 = t3p.tile([128, 3, n_in], F32, tag="p3", bufs=2)
    nc.vector.tensor_tensor(out=pr3[:, :, :], in0=in_t[:, 0:3, :],
                            in1=in_t[:, 0:3, :], op=OP.mult)
    nn = t1p.tile([128, n_in], F32, tag="s", bufs=3)
    nc.vector.tensor_tensor(out=nn[:, :], in0=pr3[:, 0, :], in1=pr3[:, 1, :],
                            op=OP.add)
    nc.vector.tensor_tensor(out=nn[:, :], in0=nn[:, :], in1=pr3[:, 2, :],
                            op=OP.add)
    nc.vector.tensor_scalar_max(out=nn[:, :], in0=nn[:, :], scalar1=1e-20)
    nc.scalar.activation(out=nn[:, :], in_=nn[:, :], func=AF.Ln)
    nc.scalar.activation(out=nn[:, :], in_=nn[:, :], func=AF.Exp, scale=-0.5)
    for c in range(3):
        nc.vector.tensor_tensor(out=in_t[:, c, :], in0=in_t[:, c, :],
                                in1=nn[:, :], op=OP.mult)

    # --- partition-aligned center copy: ctr[p] = in_t[p+10], 118 rows so the
    # extended dot rows (108+dy <= 113) stay in range
    ctr = shp.tile([128, 5, n_in], F32, tag="ctr", bufs=2)
    nc.sync.dma_start(out=ctr[0:118, :, :], in_=in_t[VPAD:128, :, :])
    ctrc = shp.tile([128, 3, n_in], F16, tag="ctrc", bufs=2)
    nc.sync.dma_start(out=ctrc[op, :, :],
                      in_=srcc_ap[VPAD:VPAD + TILE_OUT, :, :])

    # --- rdz = 1/(128*dz) on center columns (inf when dz==0; min-clamped)
    rdz = t1p.tile([128, n_out], F32, tag="rdz", bufs=2)
    nc.scalar.activation(out=rdz[op, :], in_=ctr[op, 4, of], func=AF.Ln,
                         scale=128.0)
    nc.scalar.activation(out=rdz[op, :], in_=rdz[op, :], func=AF.Exp,
                         scale=-1.0)

    # --- accumulators, initialized with the center tap (w == 1)
    accw = accp.tile([128, n_out], F32, tag="accw")
    nc.vector.memset(accw[op, :], 1.0)
    acc3 = accp.tile([128, 3, n_out], F16, tag="acc3")
    nc.vector.tensor_copy(out=acc3[op, :, :], in_=ctrc[op, :, of])

    mctx = (op, n_out, accw, acc3)

    for dy, dxs in pair_groups.items():
        u = TILE_OUT + dy              # extended dot rows
        eop = slice(0, u)
        if dy == 0:
            sh_m = ctr                 # member reads resolve against ctr
            sh_p = None
        else:
            # sh_m[p] = in_t[p+10-dy]: normals+z+colors for the -t member and
            # the shared dot/|dz| planes
            sh_m = shp.tile([128, 4, n_in], F32, tag="sh_m", bufs=2)
            nc.sync.dma_start(out=sh_m[0:u, :, :],
                              in_=in_t[VPAD - dy:VPAD - dy + u, 0:4, :])
            shc_m = shp.tile([128, 3, n_in], F16, tag="shc_m", bufs=2)
            nc.sync.dma_start(out=shc_m[op, :, :],
                              in_=srcc_ap[VPAD - dy:VPAD - dy + TILE_OUT,
                                          :, :])
            # +t member colors
            shc_p = shp.tile([128, 3, n_in], F16, tag="shc_p", bufs=2)
            nc.sync.dma_start(out=shc_p[op, :, :],
                              in_=srcc_ap[VPAD + dy:VPAD + dy + TILE_OUT,
                                          :, :])
        gacc = accp.tile([128, 3, n_out], F16, tag="gacc", bufs=2)
        gaccw = accp.tile([128, n_out], F16, tag="gaccw", bufs=2)
        nc.vector.memset(gaccw[op, :], 0.0)
        first = True
        rm = None
        last_adx = None
        for dx in dxs:
            d2 = dy * dy + dx * dx
            if abs(dx) != last_adx:
                last_adx = abs(dx)
                rm = rmp.tile([128, n_out], F16, tag="rm", bufs=2)
                nc.vector.tensor_scalar(out=rm[op, :], in0=rdz[op, :],
                                        scalar1=1.0 / math.sqrt(d2),
                                        scalar2=RM_CLAMP,
                                        op0=OP.mult, op1=OP.min)
            bias_ap = bias_tile[op, cls_idx[d2]:cls_idx[d2] + 1]
            lo = max(0, -dx)
            hi = n_in - max(0, dx)
            # shared planes: P2[.,0,x] = dot(n(r), n(r+t)) at base row
            # r = p+10-dy;  P2[.,1,x] = |z(r+t) - z(r)|
            pr = t3p.tile([128, 3, n_in], F32, tag="p3", bufs=2)
            nc.vector.tensor_tensor(out=pr[eop, :, lo:hi],
                                    in0=sh_m[eop, 0:3, lo:hi],
                                    in1=ctr[eop, 0:3, lo + dx:hi + dx],
                                    op=OP.mult)
            pL = shp.tile([128, n_in], F32, tag="pL", bufs=2)
            nc.vector.tensor_tensor(out=pL[eop, lo:hi],
                                    in0=pr[eop, 0, lo:hi],
                                    in1=pr[eop, 1, lo:hi], op=OP.add)
            nc.vector.tensor_tensor(out=pL[eop, lo:hi],
                                    in0=pL[eop, lo:hi],
                                    in1=pr[eop, 2, lo:hi], op=OP.add)
            pA = shp.tile([128, n_in], F16, tag="pA", bufs=2)
            nc.vector.tensor_tensor(out=pA[eop, lo:hi],
                                    in0=ctr[eop, 3, lo + dx:hi + dx],
                                    in1=sh_m[eop, 3, lo:hi], op=OP.subtract)
            nc.scalar.activation(out=pL[eop, lo:hi], in_=pL[eop, lo:hi],
                                 func=AF.Ln)
            nc.scalar.activation(out=pA[eop, lo:hi], in_=pA[eop, lo:hi],
                                 func=AF.Abs)
            # member -t = (-dy,-dx): aligned rows, columns shifted by -dx
            sm = slice(RAD - dx, RAD - dx + n_out)
            col_m = (ctrc[op, :, sm] if dy == 0 else shc_m[op, :, sm])
            _emit_member(nc, (t1p, t3p), mctx,
                         pL[op, sm], pA[op, sm], col_m,
                         rm[op, :], bias_ap, gacc, gaccw, first)
            first = False
            # member +t = (dy,dx): rows shifted by +dy (DMA copy when dy>0)
            sfp = slice(RAD + dx, RAD + dx + n_out)
            if dy == 0:
                lnu_p, azd_p = pL[op, of], pA[op, of]
                col_p = ctrc[op, :, sfp]
            else:
                pLs = shp.tile([128, n_out], F32, tag="pLs", bufs=2)
                nc.sync.dma_start(out=pLs[op, :],
                                  in_=pL[dy:dy + TILE_OUT, of])
                # f16 shift copy with a 4B-aligned source offset (cols 4..)
                pAs = shp.tile([128, n_out + 2], F16, tag="pAs", bufs=2)
                nc.sync.dma_start(out=pAs[op, :],
                                  in_=pA[dy:dy + TILE_OUT,
                                         RAD - 1:RAD + 1 + n_out])
                lnu_p, azd_p = pLs[op, :], pAs[op, 1:1 + n_out]
                col_p = shc_p[op, :, sfp]
            _emit_member(nc, (t1p, t3p), mctx,
                         lnu_p, azd_p, col_p, rm[op, :], bias_ap, gacc,
                         gaccw, False)
        nc.vector.tensor_tensor(out=acc3[op, :, :], in0=acc3[op, :, :],
                                in1=gacc[op, :, :], op=OP.add)
        nc.vector.tensor_tensor(out=accw[op, :], in0=accw[op, :],
                                in1=gaccw[op, :], op=OP.add)

    # --- out = acc3 / accw
    nc.vector.reciprocal(out=accw[op, :], in_=accw[op, :])
    rwh = t1p.tile([128, n_out], F16, tag="rwh", bufs=2)
    nc.vector.tensor_copy(out=rwh[op, :], in_=accw[op, :])
    out3 = t3p.tile([128, 3, n_out], F32, tag="p3", bufs=2)
    rw_b = rwh[op, None, :].to_broadcast((TILE_OUT, 3, n_out))
    nc.vector.tensor_tensor(out=out3[op, :, :], in0=acc3[op, :, :], in1=rw_b,
                            op=OP.mult)
    nc.sync.dma_start(out=dst_ap, in_=out3[op, :, :])


def _build(tensors, items, classes):
    """tensors: {name: (shape, kind)}; items: (in_name, col0, n_in, out_name,
    out_col0)."""
    nc = bacc.Bacc(None)
    handles = {}
    for name, (shape, kind) in tensors.items():
        dt = F16 if name.endswith("c") else F32
        handles[name] = nc.dram_tensor(name, list(shape), dt, kind=kind)
    cls_idx = {d2: k for k, (d2, _) in enumerate(classes)}
    # Preload the one act-table set containing Ln+Exp+Abs so the compiler's
    # per-activation table-load pass (first-containing-set policy) doesn't
    # thrash between the ln-only and exp-only sets on every tap.
    from concourse.hw_specs import get_activation_tables
    _tables = get_activation_tables(nc.m.arch)
    _need = {AF.Ln, AF.Exp, AF.Abs}
    _combined = next(i for i, (_, fs) in enumerate(_tables.items())
                     if _need <= fs)
    with tile.TileContext(nc) as tc:
        nc.scalar.add_instruction(mybir.InstLoadActFuncSet(
            act_func_set_id=_combined,
            name=nc.get_next_instruction_name(),
            engine=nc.scalar.engine,
            ins=[], outs=[]))
        with (
            tc.tile_pool(name="inp", bufs=1) as inp,
            tc.tile_pool(name="sh", bufs=1) as shp,
            tc.tile_pool(name="t3", bufs=1) as t3p,
            tc.tile_pool(name="t1", bufs=1) as t1p,
            tc.tile_pool(name="acc", bufs=1) as accp,
            tc.tile_pool(name="rm", bufs=1) as rmp,
            tc.tile_pool(name="bias", bufs=1) as biasp,
        ):
            bias_tile = biasp.tile([128, len(classes)], F32)
            for d2, k in cls_idx.items():
                nc.vector.memset(bias_tile[:, k:k + 1], -d2 / 8.0)
            pools = (inp, shp, t3p, t1p, accp, rmp)
            for in_name, col0, n_in, out_name, out_col0 in items:
                n_out = n_in - 2 * RAD
                src = handles[in_name][:, :, col0:col0 + n_in]
                src = src.rearrange("c h w -> h c w")
                srcc = handles[in_name + "c"][:, :, col0:col0 + n_in]
                srcc = srcc.rearrange("c h w -> h c w")
                dst = handles[out_name][:, :, out_col0:out_col0 + n_out]
                dst = dst.rearrange("c h w -> h c w")
                _emit_item(nc, pools, src, srcc, dst, n_in, classes,
                           bias_tile, cls_idx)
    nc.finalize()
    return nc


_CACHE = {}


def _get_full():
    if "full" not in _CACHE:
        tensors = {
            "xa": ((5, 128, W + 10), "ExternalInput"),
            "xac": ((3, 128, W + 10), "ExternalInput"),
            "xb": ((5, 128, 490), "ExternalInput"),
            "xbc": ((3, 128, 490), "ExternalInput"),
            "ya": ((3, TILE_OUT, W), "ExternalOutput"),
            "yb": ((3, TILE_OUT, 480), "ExternalOutput"),
        }
        items = [
            ("xa", 0, 650, "ya", 0),
            ("xa", 640, 650, "ya", 640),
            ("xa", 1280, 650, "ya", 1280),
            ("xb", 0, 490, "yb", 0),
        ]
        _CACHE["full"] = _build(tensors, items, tap_classes())
    return _CACHE["full"]


def _get_mini(n_in=202, n_classes=None):
    key = ("mini", n_in, n_classes)
    classes = tap_classes()
    if n_classes is not None:
        classes = classes[:n_classes]
    if key not in _CACHE:
        n_out = n_in - 2 * RAD
        tensors = {
            "xm": ((5, 128, n_in), "ExternalInput"),
            "xmc": ((3, 128, n_in), "ExternalInput"),
            "ym": ((3, TILE_OUT, n_out), "ExternalOutput"),
        }
        items = [("xm", 0, n_in, "ym", 0)]
        _CACHE[key] = _build(tensors, items, classes)
    return _CACHE[key], classes


def _make_planes(inp):
    """[H,W,8] -> padded f32 planes [5,...] + padded fp16 colors [3,...]."""
    src = np.moveaxis(np.asarray(inp, dtype=np.float32), -1, 0)
    h, w = src.shape[1], src.shape[2]
    planes = np.zeros((5, h + 2 * VPAD, w + 2 * RAD), np.float32)
    planes[:, VPAD:VPAD + h, RAD:RAD + w] = src[PLANE_PERM]
    cols = np.zeros((3, h + 2 * VPAD, w + 2 * RAD), np.float16)
    cols[:, VPAD:VPAD + h, RAD:RAD + w] = src[COL_PERM]
    return planes, cols


LAST_RESULTS = None


def kernel(input, _trace=False):
    global LAST_RESULTS
    inp = np.asarray(input, dtype=np.float32)[0]          # [1080, 1920, 8]
    planes, cols = _make_planes(inp)                      # [5|3, 1100, 1930]
    T = TILE_OUT
    in_maps = []
    for i in range(8):
        xa = np.ascontiguousarray(planes[:, T * i:T * i + 128, :])
        xac = np.ascontiguousarray(cols[:, T * i:T * i + 128, :])
        if i < 4:
            r0 = 8 * T          # tile-row 8: output rows [864, 972)
            c0 = 480 * i
        else:
            r0 = 9 * T          # tile-row 9: output rows [972, 1080)
            c0 = 480 * (i - 4)
        xb = np.ascontiguousarray(planes[:, r0:r0 + 128, c0:c0 + 490])
        xbc = np.ascontiguousarray(cols[:, r0:r0 + 128, c0:c0 + 490])
        in_maps.append({"xa": xa, "xac": xac, "xb": xb, "xbc": xbc})
    nc = _get_full()
    res = run_bass_kernel_spmd(nc, in_maps, core_ids=list(range(8)),
                               trace=_trace)
    LAST_RESULTS = res
    out = np.empty((H, W, 3), np.float32)
    for i in range(8):
        out[T * i:T * i + T] = np.moveaxis(res.results[i]["ya"], 0, -1)
    for i in range(8):
        yb = np.moveaxis(res.results[i]["yb"], 0, -1)
        if i < 4:
            out[8 * T:9 * T, 480 * i:480 * i + 480] = yb
        else:
            j = i - 4
            out[9 * T:10 * T, 480 * j:480 * j + 480] = yb
    return out[None]


# revision 20
# speedup vs baseline: 1.0087x; 1.0087x over previous
"""Bilateral denoiser (11x11 window, sigma=2) on 8 Trainium2 NeuronCores.

Math (per output pixel p, tap offset t=(dy,dx), d2 = dy^2+dx^2):
    w_t = exp(128*ln(dot(nrm_p, nrm_{p+t})) - |z_{p+t}-z_p|*min(1/(dz_p*c), 1e4)
              - d2/8)
        = clip(dot,0,1)^128 * exp(-|dz|/max(dz*sqrt(d2), 1e-4)) * exp(-d2/8)
    out = sum_t w_t * col_{p+t} / sum_t w_t     (center tap has w=1)

Sharding: H=1080 rows -> 10 tile rows of exactly 108 output rows, each
computed from a 128-row input tile (+-10-row halo in the partition dim).
Every core gets tile-row i full-width plus one 480-wide strip of tile-rows
8/9, so all cores run an identical (SPMD) program on identical shapes.

Symmetry: dot(n_p, n_{p+t}) and |z_{p+t}-z_p| are symmetric in (p, p+t), so
each pair {t, -t} shares one dot/|dz| plane computed on 108+dy extended rows;
the +t member reads it through a partition-shifted DMA copy (engine access
patterns must start at partition 0 -- quadrant rule -- so shifts go via DMA).

Host side only pads/deinterleaves/slices (layout, no math); all arithmetic
runs on-device: DVE tensor ops + ScalarE Ln/Exp/Abs (one act table set).
"""
import math

import numpy as np

import concourse.bass as bass
import concourse.bacc as bacc
import concourse.tile as tile
from concourse import mybir
from concourse.bass_utils import run_bass_kernel_spmd

F32 = mybir.dt.float32
F16 = mybir.dt.float16
AF = mybir.ActivationFunctionType
OP = mybir.AluOpType

RAD = 5
H, W = 1080, 1920
TILE_OUT = 108            # output rows per 128-partition tile (2*RAD halo x2)
VPAD = 2 * RAD            # vertical halo rows above/below each tile
RM_CLAMP = 1.0 / (128.0 * 1e-4)

# f32 plane order: 0:3 = normal(xyz), 3 = z, 4 = dz;  colors ride separately
# as fp16 planes (halves the DVE cost of the color multiply/accumulate).
PLANE_PERM = [3, 4, 5, 6, 7]  # input channels -> f32 planes
COL_PERM = [0, 1, 2]          # input channels -> f16 color planes


def tap_classes(rad=RAD):
    cls = {}
    for dy in range(-rad, rad + 1):
        for dx in range(-rad, rad + 1):
            if dy == 0 and dx == 0:
                continue
            cls.setdefault(dy * dy + dx * dx, []).append((dy, dx))
    return sorted(cls.items())


def _pairs(classes):
    """Pairs {(dy,dx), (-dy,-dx)} grouped by dy >= 0; rep has dy>0 or
    (dy==0 and dx>0). Returns {dy: [dx,...]} honoring the class subset."""
    tap_set = {t for _, taps in classes for t in taps}
    groups = {}
    for dy in range(0, RAD + 1):
        dxs = []
        for dx in range(-RAD, RAD + 1):
            if dy == 0 and dx <= 0:
                continue
            if (dy, dx) in tap_set:
                dxs.append(dx)
        if dxs:
            # order by |dx| so +-dx neighbors share the rm plane
            dxs.sort(key=lambda d: (abs(d), -d))
            groups[dy] = dxs
    return groups


def _pair2(base, step):
    """Insert an outer count-2 free dim (element stride ``step``) after the
    partition dim of ``base`` -- two sibling planes in one access pattern."""
    return bass.AP(tensor=base.tensor, offset=base.offset,
                   ap=[base.ap[0], [step, 2]] + list(base.ap[1:]))


def _emit_member(nc, pools, ctx, lnu_ap, azd_ap, col_ap, rm_ap, bias_ap,
                 gacc, gaccw, first):
    """Accumulate one tap given its aligned ln(dot), |dz|, color APs.

    Color contributions go into the per-dy-group fp16 partial ``gacc``
    (small magnitudes -> small fp16 rounding); the first member writes it
    directly."""
    t1p, t3p = pools
    op, n_out, accw, acc3 = ctx
    # t1 = |dz|*rm in fp16 (2x DVE mode); x = lnu - t1 back in f32 so the
    # 128x-amplified exponent keeps full precision.
    t1 = t1p.tile([128, n_out], F16, tag="t1h", bufs=3)
    nc.vector.tensor_tensor(out=t1[op, :], in0=azd_ap, in1=rm_ap, op=OP.mult)
    xx = t1p.tile([128, n_out], F32, tag="s", bufs=3)
    nc.vector.tensor_tensor(out=xx[op, :], in0=lnu_ap, in1=t1[op, :],
                            op=OP.subtract)
    # w in fp16: the same rounded w feeds numerator and denominator, so the
    # rounding largely cancels in the final ratio.
    wt = t1p.tile([128, n_out], F16, tag="w", bufs=3)
    nc.scalar.activation(out=wt[op, :], in_=xx[op, :], func=AF.Exp,
                         scale=128.0, bias=bias_ap)
    nc.vector.tensor_tensor(out=gaccw[op, :], in0=gaccw[op, :],
                            in1=wt[op, :], op=OP.add)
    w_b = wt[op, None, :].to_broadcast((TILE_OUT, 3, n_out))
    if first:
        nc.vector.tensor_tensor(out=gacc[op, :, :], in0=col_ap, in1=w_b,
                                op=OP.mult)
    else:
        wc = t3p.tile([128, 3, n_out], F16, tag="p3c", bufs=2)
        nc.vector.tensor_tensor(out=wc[op, :, :], in0=col_ap, in1=w_b,
                                op=OP.mult)
        nc.vector.tensor_tensor(out=gacc[op, :, :], in0=gacc[op, :, :],
                                in1=wc[op, :, :], op=OP.add)


def _emit_item(nc, pools, src_ap, srcc_ap, dst_ap, n_in, classes, bias_tile,
               cls_idx):
    """One work item: input planes [8,128,n_in] -> output [108,3,n_in-10]."""
    n_out = n_in - 2 * RAD
    inp, shp, t3p, t1p, accp, rmp = pools
    op = slice(0, TILE_OUT)            # output rows = in_t rows [10,118)
    of = slice(RAD, RAD + n_out)       # center free columns within n_in

    pair_groups = _pairs(classes)

    in_t = inp.tile([128, 5, n_in], F32, tag="in", bufs=2)
    nc.sync.dma_start(out=in_t[:, :, :], in_=src_ap)

    # --- normalize normals in place: n *= (max(|n|^2,1e-20))^-0.5
    pr3 = t3p.tile([128, 3, n_in], F32, tag="p3", bufs=2)
    nc.vector.tensor_tensor(out=pr3[:, :, :], in0=in_t[:, 0:3, :],
                            in1=in_t[:, 0:3, :], op=OP.mult)
    nn = t1p.tile([128, n_in], F32, tag="s", bufs=3)
    nc.vector.tensor_tensor(out=nn[:, :], in0=pr3[:, 0, :], in1=pr3[:, 1, :],
                            op=OP.add)
    nc.vector.tensor_tensor(out=nn[:, :], in0=nn[:, :], in1=pr3[:, 2, :],
                            op=OP.add)
    nc.vector.tensor_scalar_max(out=nn[:, :], in0=nn[:, :], scalar1=1e-20)
    nc.scalar.activation(out=nn[:, :], in_=nn[:, :], func=AF.Ln)
    nc.scalar.activation(out=nn[:, :], in_=nn[:, :], func=AF.Exp, scale=-0.5)
    for c in range(3):
        nc.vector.tensor_tensor(out=in_t[:, c, :], in0=in_t[:, c, :],
                                in1=nn[:, :], op=OP.mult)

    # --- partition-aligned center copy: ctr[p] = in_t[p+10], 118 rows so the
    # extended dot rows (108+dy <= 113) stay in range
    ctr = shp.tile([128, 5, n_in], F32, tag="ctr", bufs=2)
    nc.sync.dma_start(out=ctr[0:118, :, :], in_=in_t[VPAD:128, :, :])
    ctrc = shp.tile([128, 3, n_in], F16, tag="ctrc", bufs=2)
    nc.sync.dma_start(out=ctrc[op, :, :],
                      in_=srcc_ap[VPAD:VPAD + TILE_OUT, :, :])

    # --- rdz = 1/(128*dz) on center columns (inf when dz==0; min-clamped)
    rdz = t1p.tile([128, n_out], F32, tag="rdz", bufs=2)
    nc.scalar.activation(out=rdz[op, :], in_=ctr[op, 4, of], func=AF.Ln,
                         scale=128.0)
    nc.scalar.activation(out=rdz[op, :], in_=rdz[op, :], func=AF.Exp,
                         scale=-1.0)

    # --- accumulators, initialized with the center tap (w == 1)
    accw = accp.tile([128, n_out], F32, tag="accw")
    nc.vector.memset(accw[op, :], 1.0)
    acc3 = accp.tile([128, 3, n_out], F16, tag="acc3")
    nc.vector.tensor_copy(out=acc3[op, :, :], in_=ctrc[op, :, of])

    mctx = (op, n_out, accw, acc3)

    for dy, dxs in pair_groups.items():
        u = TILE_OUT + dy              # extended dot rows
        eop = slice(0, u)
        if dy == 0:
            sh_m = ctr                 # member reads resolve against ctr
            sh_p = None
        else:
            # sh_m[p] = in_t[p+10-dy]: normals+z+colors for the -t member and
            # the shared dot/|dz| planes
            sh_m = shp.tile([128, 4, n_in], F32, tag="sh_m", bufs=2)
            nc.sync.dma_start(out=sh_m[0:u, :, :],
                              in_=in_t[VPAD - dy:VPAD - dy + u, 0:4, :])
            shc_m = shp.tile([128, 3, n_in], F16, tag="shc_m", bufs=2)
            nc.sync.dma_start(out=shc_m[op, :, :],
                              in_=srcc_ap[VPAD - dy:VPAD - dy + TILE_OUT,
                                          :, :])
            # +t member colors
            shc_p = shp.tile([128, 3, n_in], F16, tag="shc_p", bufs=2)
            nc.sync.dma_start(out=shc_p[op, :, :],
                              in_=srcc_ap[VPAD + dy:VPAD + dy + TILE_OUT,
                                          :, :])
        gacc = accp.tile([128, 3, n_out], F16, tag="gacc", bufs=2)
        gaccw = accp.tile([128, n_out], F16, tag="gaccw", bufs=2)
        nc.vector.memset(gaccw[op, :], 0.0)
        first = True
        rm = None
        last_adx = None
        last_merged = None
        for dx in dxs:
            d2 = dy * dy + dx * dx
            if abs(dx) != last_adx:
                last_adx = abs(dx)
                rm = rmp.tile([128, n_out], F16, tag="rm", bufs=2)
                nc.vector.tensor_scalar(out=rm[op, :], in0=rdz[op, :],
                                        scalar1=1.0 / math.sqrt(d2),
                                        scalar2=RM_CLAMP,
                                        op0=OP.mult, op1=OP.min)
            if abs(dx) != last_merged and dy > 0 and dx > 0 and -dx in dxs:
                # one fused emission for BOTH pairs {+dx} (plane 0, cols
                # [0,wD)) and {-dx} (plane 1, cols [dx, dx+wD)): the pair
                # dim rides as an outer count-2 AP dim.
                last_merged = dx
                adx = dx
                wD = n_in - adx
                pr4 = t3p.tile([128, 2, 3, n_in], F32, tag="p4", bufs=1)
                pL2 = shp.tile([128, 2, n_in], F32, tag="pL", bufs=2)
                pA2 = shp.tile([128, 2, n_in], F16, tag="pA", bufs=2)
                nc.vector.tensor_tensor(
                    out=_pair2(pr4[eop, 0, :, 0:wD], 3 * n_in + adx),
                    in0=_pair2(sh_m[eop, 0:3, 0:wD], adx),
                    in1=_pair2(ctr[eop, 0:3, adx:adx + wD], -adx),
                    op=OP.mult)
                nc.vector.tensor_tensor(
                    out=_pair2(pL2[eop, 0, 0:wD], n_in + adx),
                    in0=_pair2(pr4[eop, 0, 0, 0:wD], 3 * n_in + adx),
                    in1=_pair2(pr4[eop, 0, 1, 0:wD], 3 * n_in + adx),
                    op=OP.add)
                nc.vector.tensor_tensor(
                    out=_pair2(pL2[eop, 0, 0:wD], n_in + adx),
                    in0=_pair2(pL2[eop, 0, 0:wD], n_in + adx),
                    in1=_pair2(pr4[eop, 0, 2, 0:wD], 3 * n_in + adx),
                    op=OP.add)
                nc.vector.tensor_tensor(
                    out=_pair2(pA2[eop, 0, 0:wD], n_in + adx),
                    in0=_pair2(ctr[eop, 3, adx:adx + wD], -adx),
                    in1=_pair2(sh_m[eop, 3, 0:wD], adx),
                    op=OP.subtract)
                nc.scalar.activation(
                    out=_pair2(pL2[eop, 0, 0:wD], n_in + adx),
                    in_=_pair2(pL2[eop, 0, 0:wD], n_in + adx), func=AF.Ln)
                nc.scalar.activation(
                    out=_pair2(pA2[eop, 0, 0:wD], n_in + adx),
                    in_=_pair2(pA2[eop, 0, 0:wD], n_in + adx), func=AF.Abs)
            bias_ap = bias_tile[op, cls_idx[d2]:cls_idx[d2] + 1]
            if dy > 0 and abs(dx) > 0 and -dx in dxs:
                pL = pL2[:, 0 if dx > 0 else 1, :]
                pA = pA2[:, 0 if dx > 0 else 1, :]
            else:
                lo = max(0, -dx)
                hi = n_in - max(0, dx)
                pr = t3p.tile([128, 3, n_in], F32, tag="p3", bufs=2)
                nc.vector.tensor_tensor(out=pr[eop, :, lo:hi],
                                        in0=sh_m[eop, 0:3, lo:hi],
                                        in1=ctr[eop, 0:3, lo + dx:hi + dx],
                                        op=OP.mult)
                pL = shp.tile([128, n_in], F32, tag="pL1", bufs=2)
                nc.vector.tensor_tensor(out=pL[eop, lo:hi],
                                        in0=pr[eop, 0, lo:hi],
                                        in1=pr[eop, 1, lo:hi], op=OP.add)
                nc.vector.tensor_tensor(out=pL[eop, lo:hi],
                                        in0=pL[eop, lo:hi],
                                        in1=pr[eop, 2, lo:hi], op=OP.add)
                pA = shp.tile([128, n_in], F16, tag="pA1", bufs=2)
                nc.vector.tensor_tensor(out=pA[eop, lo:hi],
                                        in0=ctr[eop, 3, lo + dx:hi + dx],
                                        in1=sh_m[eop, 3, lo:hi],
                                        op=OP.subtract)
                nc.scalar.activation(out=pL[eop, lo:hi], in_=pL[eop, lo:hi],
                                     func=AF.Ln)
                nc.scalar.activation(out=pA[eop, lo:hi], in_=pA[eop, lo:hi],
                                     func=AF.Abs)
            # member -t = (-dy,-dx): aligned rows, columns shifted by -dx
            sm = slice(RAD - dx, RAD - dx + n_out)
            col_m = (ctrc[op, :, sm] if dy == 0 else shc_m[op, :, sm])
            _emit_member(nc, (t1p, t3p), mctx,
                         pL[op, sm], pA[op, sm], col_m,
                         rm[op, :], bias_ap, gacc, gaccw, first)
            first = False
            # member +t = (dy,dx): rows shifted by +dy (DMA copy when dy>0)
            sfp = slice(RAD + dx, RAD + dx + n_out)
            if dy == 0:
                lnu_p, azd_p = pL[op, of], pA[op, of]
                col_p = ctrc[op, :, sfp]
            else:
                pLs = shp.tile([128, n_out], F32, tag="pLs", bufs=2)
                nc.sync.dma_start(out=pLs[op, :],
                                  in_=pL[dy:dy + TILE_OUT, of])
                # f16 shift copy with a 4B-aligned source offset (cols 4..)
                pAs = shp.tile([128, n_out + 2], F16, tag="pAs", bufs=2)
                nc.sync.dma_start(out=pAs[op, :],
                                  in_=pA[dy:dy + TILE_OUT,
                                         RAD - 1:RAD + 1 + n_out])
                lnu_p, azd_p = pLs[op, :], pAs[op, 1:1 + n_out]
                col_p = shc_p[op, :, sfp]
            _emit_member(nc, (t1p, t3p), mctx,
                         lnu_p, azd_p, col_p, rm[op, :], bias_ap, gacc,
                         gaccw, False)
        nc.vector.tensor_tensor(out=acc3[op, :, :], in0=acc3[op, :, :],
                                in1=gacc[op, :, :], op=OP.add)
        nc.vector.tensor_tensor(out=accw[op, :], in0=accw[op, :],
                                in1=gaccw[op, :], op=OP.add)

    # --- out = acc3 / accw
    nc.vector.reciprocal(out=accw[op, :], in_=accw[op, :])
    rwh = t1p.tile([128, n_out], F16, tag="rwh", bufs=2)
    nc.vector.tensor_copy(out=rwh[op, :], in_=accw[op, :])
    out3 = t3p.tile([128, 3, n_out], F32, tag="p3", bufs=2)
    rw_b = rwh[op, None, :].to_broadcast((TILE_OUT, 3, n_out))
    nc.vector.tensor_tensor(out=out3[op, :, :], in0=acc3[op, :, :], in1=rw_b,
                            op=OP.mult)
    nc.sync.dma_start(out=dst_ap, in_=out3[op, :, :])


def _build(tensors, items, classes):
    """tensors: {name: (shape, kind)}; items: (in_name, col0, n_in, out_name,
    out_col0)."""
    nc = bacc.Bacc(None)
    handles = {}
    for name, (shape, kind) in tensors.items():
        dt = F16 if name.endswith("c") else F32
        handles[name] = nc.dram_tensor(name, list(shape), dt, kind=kind)
    cls_idx = {d2: k for k, (d2, _) in enumerate(classes)}
    # Preload the one act-table set containing Ln+Exp+Abs so the compiler's
    # per-activation table-load pass (first-containing-set policy) doesn't
    # thrash between the ln-only and exp-only sets on every tap.
    from concourse.hw_specs import get_activation_tables
    _tables = get_activation_tables(nc.m.arch)
    _need = {AF.Ln, AF.Exp, AF.Abs}
    _combined = next(i for i, (_, fs) in enumerate(_tables.items())
                     if _need <= fs)
    with tile.TileContext(nc) as tc:
        nc.scalar.add_instruction(mybir.InstLoadActFuncSet(
            act_func_set_id=_combined,
            name=nc.get_next_instruction_name(),
            engine=nc.scalar.engine,
            ins=[], outs=[]))
        with (
            tc.tile_pool(name="inp", bufs=1) as inp,
            tc.tile_pool(name="sh", bufs=1) as shp,
            tc.tile_pool(name="t3", bufs=1) as t3p,
            tc.tile_pool(name="t1", bufs=1) as t1p,
            tc.tile_pool(name="acc", bufs=1) as accp,
            tc.tile_pool(name="rm", bufs=1) as rmp,
            tc.tile_pool(name="bias", bufs=1) as biasp,
        ):
            bias_tile = biasp.tile([128, len(classes)], F32)
            for d2, k in cls_idx.items():
                nc.vector.memset(bias_tile[:, k:k + 1], -d2 / 8.0)
            pools = (inp, shp, t3p, t1p, accp, rmp)
            for in_name, col0, n_in, out_name, out_col0 in items:
                n_out = n_in - 2 * RAD
                src = handles[in_name][:, :, col0:col0 + n_in]
                src = src.rearrange("c h w -> h c w")
                srcc = handles[in_name + "c"][:, :, col0:col0 + n_in]
                srcc = srcc.rearrange("c h w -> h c w")
                dst = handles[out_name][:, :, out_col0:out_col0 + n_out]
                dst = dst.rearrange("c h w -> h c w")
                _emit_item(nc, pools, src, srcc, dst, n_in, classes,
                           bias_tile, cls_idx)
    nc.finalize()
    return nc


_CACHE = {}


def _get_full():
    if "full" not in _CACHE:
        tensors = {
            "xa": ((5, 128, W + 10), "ExternalInput"),
            "xac": ((3, 128, W + 10), "ExternalInput"),
            "xb": ((5, 128, 490), "ExternalInput"),
            "xbc": ((3, 128, 490), "ExternalInput"),
            "ya": ((3, TILE_OUT, W), "ExternalOutput"),
            "yb": ((3, TILE_OUT, 480), "ExternalOutput"),
        }
        items = [
            ("xa", 0, 650, "ya", 0),
            ("xa", 640, 650, "ya", 640),
            ("xa", 1280, 650, "ya", 1280),
            ("xb", 0, 490, "yb", 0),
        ]
        _CACHE["full"] = _build(tensors, items, tap_classes())
    return _CACHE["full"]


def _get_mini(n_in=202, n_classes=None):
    key = ("mini", n_in, n_classes)
    classes = tap_classes()
    if n_classes is not None:
        classes = classes[:n_classes]
    if key not in _CACHE:
        n_out = n_in - 2 * RAD
        tensors = {
            "xm": ((5, 128, n_in), "ExternalInput"),
            "xmc": ((3, 128, n_in), "ExternalInput"),
            "ym": ((3, TILE_OUT, n_out), "ExternalOutput"),
        }
        items = [("xm", 0, n_in, "ym", 0)]
        _CACHE[key] = _build(tensors, items, classes)
    return _CACHE[key], classes


def _make_planes(inp):
    """[H,W,8] -> padded f32 planes [5,...] + padded fp16 colors [3,...]."""
    src = np.moveaxis(np.asarray(inp, dtype=np.float32), -1, 0)
    h, w = src.shape[1], src.shape[2]
    planes = np.zeros((5, h + 2 * VPAD, w + 2 * RAD), np.float32)
    planes[:, VPAD:VPAD + h, RAD:RAD + w] = src[PLANE_PERM]
    cols = np.zeros((3, h + 2 * VPAD, w + 2 * RAD), np.float16)
    cols[:, VPAD:VPAD + h, RAD:RAD + w] = src[COL_PERM]
    return planes, cols


LAST_RESULTS = None


def kernel(input, _trace=False):
    global LAST_RESULTS
    inp = np.asarray(input, dtype=np.float32)[0]          # [1080, 1920, 8]
    planes, cols = _make_planes(inp)                      # [5|3, 1100, 1930]
    T = TILE_OUT
    in_maps = []
    for i in range(8):
        xa = np.ascontiguousarray(planes[:, T * i:T * i + 128, :])
        xac = np.ascontiguousarray(cols[:, T * i:T * i + 128, :])
        if i < 4:
            r0 = 8 * T          # tile-row 8: output rows [864, 972)
            c0 = 480 * i
        else:
            r0 = 9 * T          # tile-row 9: output rows [972, 1080)
            c0 = 480 * (i - 4)
        xb = np.ascontiguousarray(planes[:, r0:r0 + 128, c0:c0 + 490])
        xbc = np.ascontiguousarray(cols[:, r0:r0 + 128, c0:c0 + 490])
        in_maps.append({"xa": xa, "xac": xac, "xb": xb, "xbc": xbc})
    nc = _get_full()
    res = run_bass_kernel_spmd(nc, in_maps, core_ids=list(range(8)),
                               trace=_trace)
    LAST_RESULTS = res
    out = np.empty((H, W, 3), np.float32)
    for i in range(8):
        out[T * i:T * i + T] = np.moveaxis(res.results[i]["ya"], 0, -1)
    for i in range(8):
        yb = np.moveaxis(res.results[i]["yb"], 0, -1)
        if i < 4:
            out[8 * T:9 * T, 480 * i:480 * i + 480] = yb
        else:
            j = i - 4
            out[9 * T:10 * T, 480 * j:480 * j + 480] = yb
    return out[None]


# revision 23
# speedup vs baseline: 1.0304x; 1.0216x over previous
"""Bilateral denoiser (11x11 window, sigma=2) on 8 Trainium2 NeuronCores.

Math (per output pixel p, tap offset t=(dy,dx), d2 = dy^2+dx^2):
    w_t = exp(128*ln(dot(nrm_p, nrm_{p+t})) - |z_{p+t}-z_p|*min(1/(dz_p*c), 1e4)
              - d2/8)
        = clip(dot,0,1)^128 * exp(-|dz|/max(dz*sqrt(d2), 1e-4)) * exp(-d2/8)
    out = sum_t w_t * col_{p+t} / sum_t w_t     (center tap has w=1)

Sharding: H=1080 rows -> 10 tile rows of exactly 108 output rows, each
computed from a 128-row input tile (+-10-row halo in the partition dim).
Every core gets tile-row i full-width plus one 480-wide strip of tile-rows
8/9, so all cores run an identical (SPMD) program on identical shapes.

Symmetry: dot(n_p, n_{p+t}) and |z_{p+t}-z_p| are symmetric in (p, p+t), so
each pair {t, -t} shares one dot/|dz| plane computed on 108+dy extended rows;
the +t member reads it through a partition-shifted DMA copy (engine access
patterns must start at partition 0 -- quadrant rule -- so shifts go via DMA).

Host side only pads/deinterleaves/slices (layout, no math); all arithmetic
runs on-device: DVE tensor ops + ScalarE Ln/Exp/Abs (one act table set).
"""
import math

import numpy as np

import concourse.bass as bass
import concourse.bacc as bacc
import concourse.tile as tile
from concourse import mybir
from concourse.bass_utils import run_bass_kernel_spmd

F32 = mybir.dt.float32
F16 = mybir.dt.float16
AF = mybir.ActivationFunctionType
OP = mybir.AluOpType

RAD = 5
H, W = 1080, 1920
TILE_OUT = 108            # output rows per 128-partition tile (2*RAD halo x2)
VPAD = 2 * RAD            # vertical halo rows above/below each tile
RM_CLAMP = 1.0 / (128.0 * 1e-4)

# f32 plane order: 0:3 = normal(xyz), 3 = z, 4 = dz;  colors ride separately
# as fp16 planes (halves the DVE cost of the color multiply/accumulate).
PLANE_PERM = [3, 4, 5, 6, 7]  # input channels -> f32 planes
COL_PERM = [0, 1, 2]          # input channels -> f16 color planes


def tap_classes(rad=RAD):
    cls = {}
    for dy in range(-rad, rad + 1):
        for dx in range(-rad, rad + 1):
            if dy == 0 and dx == 0:
                continue
            cls.setdefault(dy * dy + dx * dx, []).append((dy, dx))
    return sorted(cls.items())


def _pairs(classes):
    """Pairs {(dy,dx), (-dy,-dx)} grouped by dy >= 0; rep has dy>0 or
    (dy==0 and dx>0). Returns {dy: [dx,...]} honoring the class subset."""
    tap_set = {t for _, taps in classes for t in taps}
    groups = {}
    for dy in range(0, RAD + 1):
        dxs = []
        for dx in range(-RAD, RAD + 1):
            if dy == 0 and dx <= 0:
                continue
            if (dy, dx) in tap_set:
                dxs.append(dx)
        if dxs:
            # order by |dx| so +-dx neighbors share the rm plane
            dxs.sort(key=lambda d: (abs(d), -d))
            groups[dy] = dxs
    return groups


def _pair2(base, step):
    """Insert an outer count-2 free dim (element stride ``step``) after the
    partition dim of ``base`` -- two sibling planes in one access pattern."""
    return bass.AP(tensor=base.tensor, offset=base.offset,
                   ap=[base.ap[0], [step, 2]] + list(base.ap[1:]))


def _emit_member(nc, pools, ctx, lnu_ap, azd_ap, col_ap, rm_ap, bias_ap,
                 gacc, gaccw, first):
    """Accumulate one tap given its aligned ln(dot), |dz|, color APs.

    Color contributions go into the per-dy-group fp16 partial ``gacc``
    (small magnitudes -> small fp16 rounding); the first member writes it
    directly."""
    t1p, t3p = pools
    op, n_out, accw, acc3 = ctx
    # t1 = |dz|*rm in fp16 (2x DVE mode); x = lnu - t1 back in f32 so the
    # 128x-amplified exponent keeps full precision.
    t1 = t1p.tile([128, n_out], F16, tag="t1h", bufs=2)
    nc.vector.tensor_tensor(out=t1[op, :], in0=azd_ap, in1=rm_ap, op=OP.mult)
    xx = t1p.tile([128, n_out], F32, tag="s", bufs=2)
    nc.vector.tensor_tensor(out=xx[op, :], in0=lnu_ap, in1=t1[op, :],
                            op=OP.subtract)
    # w in fp16: the same rounded w feeds numerator and denominator, so the
    # rounding largely cancels in the final ratio.
    wt = t1p.tile([128, n_out], F16, tag="w", bufs=2)
    nc.scalar.activation(out=wt[op, :], in_=xx[op, :], func=AF.Exp,
                         scale=128.0, bias=bias_ap)
    nc.vector.tensor_tensor(out=gaccw[op, :], in0=gaccw[op, :],
                            in1=wt[op, :], op=OP.add)
    w_b = wt[op, None, :].to_broadcast((TILE_OUT, 3, n_out))
    if first:
        nc.vector.tensor_tensor(out=gacc[op, :, :], in0=col_ap, in1=w_b,
                                op=OP.mult)
    else:
        wc = t3p.tile([128, 3, n_out], F16, tag="p3c", bufs=2)
        nc.vector.tensor_tensor(out=wc[op, :, :], in0=col_ap, in1=w_b,
                                op=OP.mult)
        nc.vector.tensor_tensor(out=gacc[op, :, :], in0=gacc[op, :, :],
                                in1=wc[op, :, :], op=OP.add)


def _emit_member2(nc, pools, ctx, lnu4, azd4, col4, rm_ap, bias_ap,
                  gacc, gaccw):
    """Both same-side members of the pairs {+dx},{-dx} in fused count-2 APs.
    Never the first member of a group (dx=0 runs first)."""
    t1p, t3p = pools
    op, n_out, accw, acc3 = ctx
    t14 = t1p.tile([128, 2, n_out], F16, tag="t1h", bufs=2)
    rm_b = rm_ap[op, None, :].to_broadcast((TILE_OUT, 2, n_out))
    nc.vector.tensor_tensor(out=t14[op, :, :], in0=azd4, in1=rm_b,
                            op=OP.mult)
    xx4 = t1p.tile([128, 2, n_out], F32, tag="s", bufs=2)
    nc.vector.tensor_tensor(out=xx4[op, :, :], in0=lnu4, in1=t14[op, :, :],
                            op=OP.subtract)
    wt4 = t1p.tile([128, 2, n_out], F16, tag="w", bufs=2)
    nc.scalar.activation(out=wt4[op, :, :], in_=xx4[op, :, :], func=AF.Exp,
                         scale=128.0, bias=bias_ap)
    nc.vector.tensor_tensor(out=gaccw[op, :], in0=gaccw[op, :],
                            in1=wt4[op, 0, :], op=OP.add)
    nc.vector.tensor_tensor(out=gaccw[op, :], in0=gaccw[op, :],
                            in1=wt4[op, 1, :], op=OP.add)
    wc4 = t3p.tile([128, 2, 3, n_out], F16, tag="p3c", bufs=2)
    w_b4 = wt4[op, :, None, :].to_broadcast((TILE_OUT, 2, 3, n_out))
    nc.vector.tensor_tensor(out=wc4[op, :, :, :], in0=col4, in1=w_b4,
                            op=OP.mult)
    nc.vector.tensor_tensor(out=gacc[op, :, :], in0=gacc[op, :, :],
                            in1=wc4[op, 0, :, :], op=OP.add)
    nc.vector.tensor_tensor(out=gacc[op, :, :], in0=gacc[op, :, :],
                            in1=wc4[op, 1, :, :], op=OP.add)


def _emit_item(nc, pools, src_ap, srcc_ap, dst_ap, n_in, classes, bias_tile,
               cls_idx):
    """One work item: input planes [8,128,n_in] -> output [108,3,n_in-10]."""
    n_out = n_in - 2 * RAD
    inp, shp, t3p, t1p, accp, rmp = pools
    op = slice(0, TILE_OUT)            # output rows = in_t rows [10,118)
    of = slice(RAD, RAD + n_out)       # center free columns within n_in

    pair_groups = _pairs(classes)

    in_t = inp.tile([128, 5, n_in], F32, tag="in", bufs=2)
    nc.sync.dma_start(out=in_t[:, :, :], in_=src_ap)

    # --- normalize normals in place: n *= (max(|n|^2,1e-20))^-0.5
    pr3 = t3p.tile([128, 3, n_in], F32, tag="p3", bufs=2)
    nc.vector.tensor_tensor(out=pr3[:, :, :], in0=in_t[:, 0:3, :],
                            in1=in_t[:, 0:3, :], op=OP.mult)
    nn = t1p.tile([128, n_in], F32, tag="s", bufs=2)
    nc.vector.tensor_tensor(out=nn[:, :], in0=pr3[:, 0, :], in1=pr3[:, 1, :],
                            op=OP.add)
    nc.vector.tensor_tensor(out=nn[:, :], in0=nn[:, :], in1=pr3[:, 2, :],
                            op=OP.add)
    nc.vector.tensor_scalar_max(out=nn[:, :], in0=nn[:, :], scalar1=1e-20)
    nc.scalar.activation(out=nn[:, :], in_=nn[:, :], func=AF.Ln)
    nc.scalar.activation(out=nn[:, :], in_=nn[:, :], func=AF.Exp, scale=-0.5)
    for c in range(3):
        nc.vector.tensor_tensor(out=in_t[:, c, :], in0=in_t[:, c, :],
                                in1=nn[:, :], op=OP.mult)

    # --- partition-aligned center copy: ctr[p] = in_t[p+10], 118 rows so the
    # extended dot rows (108+dy <= 113) stay in range
    ctr = shp.tile([128, 5, n_in], F32, tag="ctr", bufs=1)
    nc.sync.dma_start(out=ctr[0:118, :, :], in_=in_t[VPAD:128, :, :])
    ctrc = shp.tile([128, 3, n_in], F16, tag="ctrc", bufs=2)
    nc.sync.dma_start(out=ctrc[op, :, :],
                      in_=srcc_ap[VPAD:VPAD + TILE_OUT, :, :])

    # --- rdz = 1/(128*dz) on center columns (inf when dz==0; min-clamped)
    rdz = t1p.tile([128, n_out], F32, tag="rdz", bufs=1)
    nc.scalar.activation(out=rdz[op, :], in_=ctr[op, 4, of], func=AF.Ln,
                         scale=128.0)
    nc.scalar.activation(out=rdz[op, :], in_=rdz[op, :], func=AF.Exp,
                         scale=-1.0)

    # --- accumulators, initialized with the center tap (w == 1)
    accw = accp.tile([128, n_out], F32, tag="accw")
    nc.vector.memset(accw[op, :], 1.0)
    acc3 = accp.tile([128, 3, n_out], F16, tag="acc3")
    nc.vector.tensor_copy(out=acc3[op, :, :], in_=ctrc[op, :, of])

    mctx = (op, n_out, accw, acc3)

    for dy, dxs in pair_groups.items():
        u = TILE_OUT + dy              # extended dot rows
        eop = slice(0, u)
        if dy == 0:
            sh_m = ctr                 # member reads resolve against ctr
            sh_p = None
        else:
            # sh_m[p] = in_t[p+10-dy]: normals+z+colors for the -t member and
            # the shared dot/|dz| planes
            sh_m = shp.tile([128, 4, n_in], F32, tag="sh_m", bufs=2)
            nc.sync.dma_start(out=sh_m[0:u, :, :],
                              in_=in_t[VPAD - dy:VPAD - dy + u, 0:4, :])
            shc_m = shp.tile([128, 3, n_in], F16, tag="shc_m", bufs=2)
            nc.sync.dma_start(out=shc_m[op, :, :],
                              in_=srcc_ap[VPAD - dy:VPAD - dy + TILE_OUT,
                                          :, :])
            # +t member colors
            shc_p = shp.tile([128, 3, n_in], F16, tag="shc_p", bufs=2)
            nc.sync.dma_start(out=shc_p[op, :, :],
                              in_=srcc_ap[VPAD + dy:VPAD + dy + TILE_OUT,
                                          :, :])
        gacc = accp.tile([128, 3, n_out], F16, tag="gacc", bufs=2)
        gaccw = accp.tile([128, n_out], F16, tag="gaccw", bufs=2)
        nc.vector.memset(gaccw[op, :], 0.0)
        first = True
        rm = None
        last_adx = None
        last_merged = None
        for dx in dxs:
            d2 = dy * dy + dx * dx
            if abs(dx) != last_adx:
                last_adx = abs(dx)
                rm = rmp.tile([128, n_out], F16, tag="rm", bufs=2)
                nc.vector.tensor_scalar(out=rm[op, :], in0=rdz[op, :],
                                        scalar1=1.0 / math.sqrt(d2),
                                        scalar2=RM_CLAMP,
                                        op0=OP.mult, op1=OP.min)
            if abs(dx) != last_merged and dy > 0 and dx > 0 and -dx in dxs:
                # one fused emission for BOTH pairs {+dx} (plane 0, cols
                # [0,wD)) and {-dx} (plane 1, cols [dx, dx+wD)): the pair
                # dim rides as an outer count-2 AP dim.
                last_merged = dx
                adx = dx
                wD = n_in - adx
                pr4 = t3p.tile([128, 2, 3, n_in], F32, tag="p4", bufs=1)
                pL2 = shp.tile([128, 2, n_in], F32, tag="pL", bufs=2)
                pA2 = shp.tile([128, 2, n_in], F16, tag="pA", bufs=2)
                nc.vector.tensor_tensor(
                    out=_pair2(pr4[eop, 0, :, 0:wD], 3 * n_in + adx),
                    in0=_pair2(sh_m[eop, 0:3, 0:wD], adx),
                    in1=_pair2(ctr[eop, 0:3, adx:adx + wD], -adx),
                    op=OP.mult)
                nc.vector.tensor_tensor(
                    out=_pair2(pL2[eop, 0, 0:wD], n_in + adx),
                    in0=_pair2(pr4[eop, 0, 0, 0:wD], 3 * n_in + adx),
                    in1=_pair2(pr4[eop, 0, 1, 0:wD], 3 * n_in + adx),
                    op=OP.add)
                nc.vector.tensor_tensor(
                    out=_pair2(pL2[eop, 0, 0:wD], n_in + adx),
                    in0=_pair2(pL2[eop, 0, 0:wD], n_in + adx),
                    in1=_pair2(pr4[eop, 0, 2, 0:wD], 3 * n_in + adx),
                    op=OP.add)
                nc.vector.tensor_tensor(
                    out=_pair2(pA2[eop, 0, 0:wD], n_in + adx),
                    in0=_pair2(ctr[eop, 3, adx:adx + wD], -adx),
                    in1=_pair2(sh_m[eop, 3, 0:wD], adx),
                    op=OP.subtract)
                nc.scalar.activation(
                    out=_pair2(pL2[eop, 0, 0:wD], n_in + adx),
                    in_=_pair2(pL2[eop, 0, 0:wD], n_in + adx), func=AF.Ln)
                nc.scalar.activation(
                    out=_pair2(pA2[eop, 0, 0:wD], n_in + adx),
                    in_=_pair2(pA2[eop, 0, 0:wD], n_in + adx), func=AF.Abs)
                bias_ap = bias_tile[op, cls_idx[d2]:cls_idx[d2] + 1]
                # both -t members: plane0 cols 5-adx, plane1 cols 5+adx
                c0 = RAD - adx
                lnu4 = _pair2(pL2[op, 0, c0:c0 + n_out], n_in + 2 * adx)
                azd4 = _pair2(pA2[op, 0, c0:c0 + n_out], n_in + 2 * adx)
                col4 = _pair2(shc_m[op, 0:3, c0:c0 + n_out], 2 * adx)
                _emit_member2(nc, (t1p, t3p), mctx, lnu4, azd4, col4,
                              rm, bias_ap, gacc, gaccw)
                # both +t members via one 2-plane partition-shifted copy
                pLs4 = shp.tile([128, 2, n_out], F32, tag="pLs", bufs=2)
                nc.sync.dma_start(out=pLs4[op, :, :],
                                  in_=pL2[dy:dy + TILE_OUT, :, of])
                pAs4 = shp.tile([128, 2, n_out + 2], F16, tag="pAs", bufs=2)
                nc.sync.dma_start(
                    out=pAs4[op, :, :],
                    in_=pA2[dy:dy + TILE_OUT, :, RAD - 1:RAD + 1 + n_out])
                c1 = RAD + adx
                col4p = _pair2(shc_p[op, 0:3, c1:c1 + n_out], -2 * adx)
                _emit_member2(nc, (t1p, t3p), mctx, pLs4[op, :, :],
                              pAs4[op, :, 1:1 + n_out], col4p,
                              rm, bias_ap, gacc, gaccw)
                continue
            if dy > 0 and dx < 0 and abs(dx) == last_merged:
                continue
            bias_ap = bias_tile[op, cls_idx[d2]:cls_idx[d2] + 1]
            if False:
                pass
            else:
                lo = max(0, -dx)
                hi = n_in - max(0, dx)
                pr = t3p.tile([128, 3, n_in], F32, tag="p3", bufs=2)
                nc.vector.tensor_tensor(out=pr[eop, :, lo:hi],
                                        in0=sh_m[eop, 0:3, lo:hi],
                                        in1=ctr[eop, 0:3, lo + dx:hi + dx],
                                        op=OP.mult)
                pL = shp.tile([128, n_in], F32, tag="pL1", bufs=2)
                nc.vector.tensor_tensor(out=pL[eop, lo:hi],
                                        in0=pr[eop, 0, lo:hi],
                                        in1=pr[eop, 1, lo:hi], op=OP.add)
                nc.vector.tensor_tensor(out=pL[eop, lo:hi],
                                        in0=pL[eop, lo:hi],
                                        in1=pr[eop, 2, lo:hi], op=OP.add)
                pA = shp.tile([128, n_in], F16, tag="pA1", bufs=2)
                nc.vector.tensor_tensor(out=pA[eop, lo:hi],
                                        in0=ctr[eop, 3, lo + dx:hi + dx],
                                        in1=sh_m[eop, 3, lo:hi],
                                        op=OP.subtract)
                nc.scalar.activation(out=pL[eop, lo:hi], in_=pL[eop, lo:hi],
                                     func=AF.Ln)
                nc.scalar.activation(out=pA[eop, lo:hi], in_=pA[eop, lo:hi],
                                     func=AF.Abs)
            # member -t = (-dy,-dx): aligned rows, columns shifted by -dx
            sm = slice(RAD - dx, RAD - dx + n_out)
            col_m = (ctrc[op, :, sm] if dy == 0 else shc_m[op, :, sm])
            _emit_member(nc, (t1p, t3p), mctx,
                         pL[op, sm], pA[op, sm], col_m,
                         rm[op, :], bias_ap, gacc, gaccw, first)
            first = False
            # member +t = (dy,dx): rows shifted by +dy (DMA copy when dy>0)
            sfp = slice(RAD + dx, RAD + dx + n_out)
            if dy == 0:
                lnu_p, azd_p = pL[op, of], pA[op, of]
                col_p = ctrc[op, :, sfp]
            else:
                pLs = shp.tile([128, n_out], F32, tag="pLs", bufs=2)
                nc.sync.dma_start(out=pLs[op, :],
                                  in_=pL[dy:dy + TILE_OUT, of])
                # f16 shift copy with a 4B-aligned source offset (cols 4..)
                pAs = shp.tile([128, n_out + 2], F16, tag="pAs", bufs=2)
                nc.sync.dma_start(out=pAs[op, :],
                                  in_=pA[dy:dy + TILE_OUT,
                                         RAD - 1:RAD + 1 + n_out])
                lnu_p, azd_p = pLs[op, :], pAs[op, 1:1 + n_out]
                col_p = shc_p[op, :, sfp]
            _emit_member(nc, (t1p, t3p), mctx,
                         lnu_p, azd_p, col_p, rm[op, :], bias_ap, gacc,
                         gaccw, False)
        nc.vector.tensor_tensor(out=acc3[op, :, :], in0=acc3[op, :, :],
                                in1=gacc[op, :, :], op=OP.add)
        nc.vector.tensor_tensor(out=accw[op, :], in0=accw[op, :],
                                in1=gaccw[op, :], op=OP.add)

    # --- out = acc3 / accw
    nc.vector.reciprocal(out=accw[op, :], in_=accw[op, :])
    rwh = t1p.tile([128, n_out], F16, tag="rwh", bufs=1)
    nc.vector.tensor_copy(out=rwh[op, :], in_=accw[op, :])
    out3 = t3p.tile([128, 3, n_out], F32, tag="p3", bufs=2)
    rw_b = rwh[op, None, :].to_broadcast((TILE_OUT, 3, n_out))
    nc.vector.tensor_tensor(out=out3[op, :, :], in0=acc3[op, :, :], in1=rw_b,
                            op=OP.mult)
    nc.sync.dma_start(out=dst_ap, in_=out3[op, :, :])


def _build(tensors, items, classes):
    """tensors: {name: (shape, kind)}; items: (in_name, col0, n_in, out_name,
    out_col0)."""
    nc = bacc.Bacc(None)
    handles = {}
    for name, (shape, kind) in tensors.items():
        dt = F16 if name.endswith("c") else F32
        handles[name] = nc.dram_tensor(name, list(shape), dt, kind=kind)
    cls_idx = {d2: k for k, (d2, _) in enumerate(classes)}
    # Preload the one act-table set containing Ln+Exp+Abs so the compiler's
    # per-activation table-load pass (first-containing-set policy) doesn't
    # thrash between the ln-only and exp-only sets on every tap.
    from concourse.hw_specs import get_activation_tables
    _tables = get_activation_tables(nc.m.arch)
    _need = {AF.Ln, AF.Exp, AF.Abs}
    _combined = next(i for i, (_, fs) in enumerate(_tables.items())
                     if _need <= fs)
    with tile.TileContext(nc) as tc:
        nc.scalar.add_instruction(mybir.InstLoadActFuncSet(
            act_func_set_id=_combined,
            name=nc.get_next_instruction_name(),
            engine=nc.scalar.engine,
            ins=[], outs=[]))
        with (
            tc.tile_pool(name="inp", bufs=1) as inp,
            tc.tile_pool(name="sh", bufs=1) as shp,
            tc.tile_pool(name="t3", bufs=1) as t3p,
            tc.tile_pool(name="t1", bufs=1) as t1p,
            tc.tile_pool(name="acc", bufs=1) as accp,
            tc.tile_pool(name="rm", bufs=1) as rmp,
            tc.tile_pool(name="bias", bufs=1) as biasp,
        ):
            bias_tile = biasp.tile([128, len(classes)], F32)
            for d2, k in cls_idx.items():
                nc.vector.memset(bias_tile[:, k:k + 1], -d2 / 8.0)
            pools = (inp, shp, t3p, t1p, accp, rmp)
            for in_name, col0, n_in, out_name, out_col0 in items:
                n_out = n_in - 2 * RAD
                src = handles[in_name][:, :, col0:col0 + n_in]
                src = src.rearrange("c h w -> h c w")
                srcc = handles[in_name + "c"][:, :, col0:col0 + n_in]
                srcc = srcc.rearrange("c h w -> h c w")
                dst = handles[out_name][:, :, out_col0:out_col0 + n_out]
                dst = dst.rearrange("c h w -> h c w")
                _emit_item(nc, pools, src, srcc, dst, n_in, classes,
                           bias_tile, cls_idx)
    nc.finalize()
    return nc


_CACHE = {}


def _get_full():
    if "full" not in _CACHE:
        tensors = {
            "xa": ((5, 128, W + 10), "ExternalInput"),
            "xac": ((3, 128, W + 10), "ExternalInput"),
            "xb": ((5, 128, 490), "ExternalInput"),
            "xbc": ((3, 128, 490), "ExternalInput"),
            "ya": ((3, TILE_OUT, W), "ExternalOutput"),
            "yb": ((3, TILE_OUT, 480), "ExternalOutput"),
        }
        items = [
            ("xa", 0, 650, "ya", 0),
            ("xa", 640, 650, "ya", 640),
            ("xa", 1280, 650, "ya", 1280),
            ("xb", 0, 490, "yb", 0),
        ]
        _CACHE["full"] = _build(tensors, items, tap_classes())
    return _CACHE["full"]


def _get_mini(n_in=202, n_classes=None):
    key = ("mini", n_in, n_classes)
    classes = tap_classes()
    if n_classes is not None:
        classes = classes[:n_classes]
    if key not in _CACHE:
        n_out = n_in - 2 * RAD
        tensors = {
            "xm": ((5, 128, n_in), "ExternalInput"),
            "xmc": ((3, 128, n_in), "ExternalInput"),
            "ym": ((3, TILE_OUT, n_out), "ExternalOutput"),
        }
        items = [("xm", 0, n_in, "ym", 0)]
        _CACHE[key] = _build(tensors, items, classes)
    return _CACHE[key], classes


def _make_planes(inp):
    """[H,W,8] -> padded f32 planes [5,...] + padded fp16 colors [3,...]."""
    src = np.moveaxis(np.asarray(inp, dtype=np.float32), -1, 0)
    h, w = src.shape[1], src.shape[2]
    planes = np.zeros((5, h + 2 * VPAD, w + 2 * RAD), np.float32)
    planes[:, VPAD:VPAD + h, RAD:RAD + w] = src[PLANE_PERM]
    cols = np.zeros((3, h + 2 * VPAD, w + 2 * RAD), np.float16)
    cols[:, VPAD:VPAD + h, RAD:RAD + w] = src[COL_PERM]
    return planes, cols


LAST_RESULTS = None


def kernel(input, _trace=False):
    global LAST_RESULTS
    inp = np.asarray(input, dtype=np.float32)[0]          # [1080, 1920, 8]
    planes, cols = _make_planes(inp)                      # [5|3, 1100, 1930]
    T = TILE_OUT
    in_maps = []
    for i in range(8):
        xa = np.ascontiguousarray(planes[:, T * i:T * i + 128, :])
        xac = np.ascontiguousarray(cols[:, T * i:T * i + 128, :])
        if i < 4:
            r0 = 8 * T          # tile-row 8: output rows [864, 972)
            c0 = 480 * i
        else:
            r0 = 9 * T          # tile-row 9: output rows [972, 1080)
            c0 = 480 * (i - 4)
        xb = np.ascontiguousarray(planes[:, r0:r0 + 128, c0:c0 + 490])
        xbc = np.ascontiguousarray(cols[:, r0:r0 + 128, c0:c0 + 490])
        in_maps.append({"xa": xa, "xac": xac, "xb": xb, "xbc": xbc})
    nc = _get_full()
    res = run_bass_kernel_spmd(nc, in_maps, core_ids=list(range(8)),
                               trace=_trace)
    LAST_RESULTS = res
    out = np.empty((H, W, 3), np.float32)
    for i in range(8):
        out[T * i:T * i + T] = np.moveaxis(res.results[i]["ya"], 0, -1)
    for i in range(8):
        yb = np.moveaxis(res.results[i]["yb"], 0, -1)
        if i < 4:
            out[8 * T:9 * T, 480 * i:480 * i + 480] = yb
        else:
            j = i - 4
            out[9 * T:10 * T, 480 * j:480 * j + 480] = yb
    return out[None]
